# revision 1
# baseline (speedup 1.0000x reference)
"""LIF spike kernel for Trainium2 (Bass/Tile), data-parallel over batch on 8 cores.

Host layout per core: x_core [C=128, B_loc=4, T*HW=8192] f32 (contiguous),
so DMA-in per-partition runs are 16KB. Output uint8 [128, 4, 8192], converted
to f32 on host (spikes are exactly 0/1).

Per timestep t (on [128, 1024] column slices):
  u_t     = (neg_mem * -TAU) + x_t     DVE scalar_tensor_tensor (t=0: u_0 = x_0)
  spike_t = u_t > THRESH  (uint8)      gpsimd tensor_scalar is_gt
  neg_mem = (spike_t - 1) * u_t        DVE scalar_tensor_tensor (u8/f32 mixed)
"""

import numpy as np

import concourse.bacc as bacc
import concourse.mybir as mybir
from concourse.tile import TileContext
from concourse.bass_utils import run_bass_kernel_spmd

B, T, C, H, W = 32, 8, 128, 32, 32
HW = H * W
N_CORES = 8
B_LOC = B // N_CORES
TAU = 0.5
THRESH = 1.0

# engine per op, tunable: 'v' = vector (DVE), 'g' = gpsimd
SPIKE_ENG = ["v"] * T
NEGMEM_ENG = ["v"] * (T - 1)
U_ENG = ["v"] * (T - 1)  # index t-1 for t in 1..7

_nc_cache = None


def build_nc():
    nc = bacc.Bacc("TRN2", target_bir_lowering=False)
    f32 = mybir.dt.float32
    u8 = mybir.dt.uint8
    op = mybir.AluOpType
    x = nc.dram_tensor("x", [C, B_LOC, T * HW], f32, kind="ExternalInput")
    out = nc.dram_tensor("out", [C, B_LOC, T * HW], u8, kind="ExternalOutput")

    def eng(code):
        return nc.vector if code == "v" else nc.gpsimd

    with TileContext(nc) as tc:
        with (
            tc.tile_pool(name="xp", bufs=8) as xp,
            tc.tile_pool(name="op_", bufs=3) as opool,
            tc.tile_pool(name="up", bufs=4) as up,
            tc.tile_pool(name="mp", bufs=4) as mp,
        ):
            for b in range(B_LOC):
                xc = []
                for j in range(4):
                    xt = xp.tile([C, 2 * HW], f32, tag="xc")
                    nc.sync.dma_start(
                        out=xt[:], in_=x[:, b, j * 2 * HW : (j + 1) * 2 * HW]
                    )
                    xc.append(xt)
                ob = opool.tile([C, T * HW], u8, tag="ob")
                negmem = None
                for t in range(T):
                    xs = xc[t // 2][:, (t % 2) * HW : (t % 2 + 1) * HW]
                    if t == 0:
                        u = xs
                    else:
                        ut = up.tile([C, HW], f32, tag="u")
                        eng(U_ENG[t - 1]).scalar_tensor_tensor(
                            ut[:], negmem[:], -TAU, xs, op.mult, op.add
                        )
                        u = ut[:]
                    sp = ob[:, t * HW : (t + 1) * HW]
                    eng(SPIKE_ENG[t]).tensor_scalar(sp, u, THRESH, None, op.is_gt)
                    if t < T - 1:
                        negmem = mp.tile([C, HW], f32, tag="nm")
                        eng(NEGMEM_ENG[t]).scalar_tensor_tensor(
                            negmem[:], sp, 1.0, u, op.subtract, op.mult
                        )
                nc.sync.dma_start(out=out[:, b], in_=ob[:])
    nc.compile()
    return nc


def make_in_maps(x: np.ndarray) -> list[dict]:
    xs = np.ascontiguousarray(x).reshape(B, T, C, HW)
    return [
        {
            "x": np.ascontiguousarray(
                xs[i * B_LOC : (i + 1) * B_LOC].transpose(2, 0, 1, 3)
            ).reshape(C, B_LOC, T * HW)
        }
        for i in range(N_CORES)
    ]


def kernel(x: np.ndarray) -> np.ndarray:
    global _nc_cache
    if _nc_cache is None:
        _nc_cache = build_nc()
    res = run_bass_kernel_spmd(_nc_cache, make_in_maps(x), list(range(N_CORES)))
    # out[c, b_loc, t*HW+hw] -> [b, t, c, hw]
    parts = [
        res.results[i]["out"].reshape(C, B_LOC, T, HW).transpose(1, 2, 0, 3)
        for i in range(N_CORES)
    ]
    full = np.concatenate(parts, axis=0)
    return full.reshape(B, T, C, H, W).astype(np.float32)



# revision 3
# speedup vs baseline: 1.2578x; 1.2578x over previous
"""LIF spike kernel for Trainium2 (Bass/Tile), data-parallel over batch on 8 cores.

Host layout per core: x_core [C=128, T=8, B_loc*HW=4096] f32, so each
timestep t is one [128, 4096] tile (16KB contiguous per partition).
Output spikes uint8 [C, T, 4096], converted to f32 on host.

Math per timestep (THRESH=1, TAU=0.5), exact f32 match with reference:
  m_t = r_{t-1} * TAU + x_t          DVE scalar_tensor_tensor   (m_0 = x_0)
  s_t = sign(m_t - 1) -> u8          ACT engine; -1 saturates to 0, so
                                     s_t = (m_t > 1) exactly
  r_t = (m_t <= 1) * m_t             DVE scalar_tensor_tensor (hard reset)

The DVE chain (14 stt ops) is the bottleneck; spikes run on the otherwise
idle ACT engine, output DMA on the tensor engine's queue.
"""

import numpy as np

import concourse.bacc as bacc
import concourse.mybir as mybir
from concourse.tile import TileContext
from concourse.bass_utils import run_bass_kernel_spmd

B, T, C, H, W = 32, 8, 128, 32, 32
HW = H * W
N_CORES = 8
B_LOC = B // N_CORES
FW = B_LOC * HW  # 4096 free width per timestep tile
TAU = 0.5
THRESH = 1.0

T0_SPLIT = 4  # column chunks for t=0 head (starts compute earlier)
T7_SPLIT = 2  # column chunks for t=7 tail

_nc_cache = None


def build_nc():
    nc = bacc.Bacc("TRN2", target_bir_lowering=False)
    f32 = mybir.dt.float32
    u8 = mybir.dt.uint8
    op = mybir.AluOpType
    AF = mybir.ActivationFunctionType

    x = nc.dram_tensor("x", [C, T, FW], f32, kind="ExternalInput")
    out = nc.dram_tensor("out", [C, T, FW], u8, kind="ExternalOutput")

    with TileContext(nc) as tc:
        with (
            tc.tile_pool(name="xp", bufs=5) as xp,
            tc.tile_pool(name="x0p", bufs=1) as x0p,
            tc.tile_pool(name="mp", bufs=3) as mp,
            tc.tile_pool(name="rp", bufs=2) as rp,
            tc.tile_pool(name="sp", bufs=3) as sp,
            tc.tile_pool(name="cp", bufs=1) as cp,
        ):
            bneg1 = cp.tile([C, 1], f32)
            nc.gpsimd.memset(bneg1[:], -1.0)

            # t=0 arrives as T0_SPLIT column chunks so the chain starts early
            x0 = x0p.tile([C, FW], f32, tag="x0")
            cw = FW // T0_SPLIT
            for j in range(T0_SPLIT):
                nc.sync.dma_start(
                    out=x0[:, j * cw : (j + 1) * cw],
                    in_=x[:, 0, j * cw : (j + 1) * cw],
                )
            xts = [x0]
            for t in range(1, T):
                xt = xp.tile([C, FW], f32, tag="x")
                nc.sync.dma_start(out=xt[:], in_=x[:, t])
                xts.append(xt)

            r_prev = None
            for t in range(T):
                if t == 0:
                    m = x0
                    r_prev = rp.tile([C, FW], f32, tag="r")
                    s0 = sp.tile([C, FW], u8, tag="s")
                    for j in range(T0_SPLIT):
                        sl = slice(j * cw, (j + 1) * cw)
                        nc.vector.scalar_tensor_tensor(
                            r_prev[:, sl], m[:, sl], THRESH, m[:, sl], op.is_le, op.mult
                        )
                        nc.scalar.activation(
                            s0[:, sl], m[:, sl], AF.Sign, bias=bneg1[:], scale=1.0
                        )
                        nc.gpsimd.dma_start(out=out[:, 0, sl], in_=s0[:, sl])
                    continue

                m = mp.tile([C, FW], f32, tag="m")
                st = sp.tile([C, FW], u8, tag="s")
                if t == T - 1:
                    cw7 = FW // T7_SPLIT
                    for j in range(T7_SPLIT):
                        sl = slice(j * cw7, (j + 1) * cw7)
                        nc.vector.scalar_tensor_tensor(
                            m[:, sl], r_prev[:, sl], TAU, xts[t][:, sl], op.mult, op.add
                        )
                        nc.scalar.activation(
                            st[:, sl], m[:, sl], AF.Sign, bias=bneg1[:], scale=1.0
                        )
                        nc.gpsimd.dma_start(out=out[:, t, sl], in_=st[:, sl])
                else:
                    nc.vector.scalar_tensor_tensor(
                        m[:], r_prev[:], TAU, xts[t][:], op.mult, op.add
                    )
                    r_new = rp.tile([C, FW], f32, tag="r")
                    nc.vector.scalar_tensor_tensor(
                        r_new[:], m[:], THRESH, m[:], op.is_le, op.mult
                    )
                    nc.scalar.activation(
                        st[:], m[:], AF.Sign, bias=bneg1[:], scale=1.0
                    )
                    nc.gpsimd.dma_start(out=out[:, t], in_=st[:])
                    r_prev = r_new
    nc.compile()
    return nc


def make_in_maps(x: np.ndarray) -> list[dict]:
    # x [B, T, C, H, W] -> per core [C, T, B_loc*HW]
    xs = np.ascontiguousarray(x).reshape(B, T, C, HW)
    return [
        {
            "x": np.ascontiguousarray(
                xs[i * B_LOC : (i + 1) * B_LOC].transpose(2, 1, 0, 3)
            ).reshape(C, T, FW)
        }
        for i in range(N_CORES)
    ]


def kernel(x: np.ndarray) -> np.ndarray:
    global _nc_cache
    if _nc_cache is None:
        _nc_cache = build_nc()
    res = run_bass_kernel_spmd(_nc_cache, make_in_maps(x), list(range(N_CORES)))
    # out[c, t, b_loc*HW+hw] -> [b, t, c, hw]
    parts = [
        res.results[i]["out"].reshape(C, T, B_LOC, HW).transpose(2, 1, 0, 3)
        for i in range(N_CORES)
    ]
    full = np.concatenate(parts, axis=0)
    return full.reshape(B, T, C, H, W).astype(np.float32)


# revision 6
# speedup vs baseline: 1.3082x; 1.0401x over previous
"""LIF spike kernel for Trainium2 (Bass/Tile), data-parallel over batch on 8 cores.

Host layout per core: x_core [C=128, T=8, B_loc*HW=4096] f32, so each
timestep t is one [128, 4096] tile (16KB contiguous per partition).
Output spikes uint8 [C, T, 4096], converted to f32 on host.

Math per timestep (THRESH=1, TAU=0.5), exact f32 match with reference:
  m_t = r_{t-1} * TAU + x_t          DVE scalar_tensor_tensor   (m_0 = x_0)
  s_t = sign(m_t - 1) -> u8          ACT engine; -1 saturates to 0, so
                                     s_t = (m_t > 1) exactly
  r_t = (m_t <= 1) * m_t             DVE scalar_tensor_tensor (hard reset)

The DVE chain (14 stt ops) is the bottleneck; spikes run on the otherwise
idle ACT engine, output DMA on the tensor engine's queue.
"""

import numpy as np

import concourse.bacc as bacc
import concourse.mybir as mybir
from concourse.tile import TileContext
from concourse.bass_utils import run_bass_kernel_spmd

B, T, C, H, W = 32, 8, 128, 32, 32
HW = H * W
N_CORES = 8
B_LOC = B // N_CORES
FW = B_LOC * HW  # 4096 free width per timestep tile
TAU = 0.5
THRESH = 1.0

T0_SPLIT = 4  # column chunks for t=0 head (starts compute earlier)
T1_SPLIT = 2  # column chunks for t=1 (x_1 still streaming in)
T7_SPLIT = 4  # column chunks for t=7 tail

_nc_cache = None


def build_nc():
    nc = bacc.Bacc("TRN2", target_bir_lowering=False)
    f32 = mybir.dt.float32
    u8 = mybir.dt.uint8
    op = mybir.AluOpType
    AF = mybir.ActivationFunctionType

    x = nc.dram_tensor("x", [C, T, FW], f32, kind="ExternalInput")
    out = nc.dram_tensor("out", [C, T, FW], u8, kind="ExternalOutput")

    with TileContext(nc) as tc:
        with (
            tc.tile_pool(name="xp", bufs=5) as xp,
            tc.tile_pool(name="x0p", bufs=1) as x0p,
            tc.tile_pool(name="mp", bufs=3) as mp,
            tc.tile_pool(name="rp", bufs=2) as rp,
            tc.tile_pool(name="sp", bufs=3) as sp,
            tc.tile_pool(name="cp", bufs=1) as cp,
        ):
            bneg1 = cp.tile([C, 1], f32)
            nc.gpsimd.memset(bneg1[:], -1.0)

            # t=0 arrives as T0_SPLIT column chunks so the chain starts early
            x0 = x0p.tile([C, FW], f32, tag="x0")
            cw = FW // T0_SPLIT
            for j in range(T0_SPLIT):
                nc.sync.dma_start(
                    out=x0[:, j * cw : (j + 1) * cw],
                    in_=x[:, 0, j * cw : (j + 1) * cw],
                )
            xts = [x0]
            for t in range(1, T):
                xt = xp.tile([C, FW], f32, tag="x")
                if t == 1:
                    hw1 = FW // T1_SPLIT
                    for j in range(T1_SPLIT):
                        nc.sync.dma_start(
                            out=xt[:, j * hw1 : (j + 1) * hw1],
                            in_=x[:, 1, j * hw1 : (j + 1) * hw1],
                        )
                else:
                    nc.sync.dma_start(out=xt[:], in_=x[:, t])
                xts.append(xt)

            r_prev = None
            for t in range(T):
                if t == 0:
                    m = x0
                    r_prev = rp.tile([C, FW], f32, tag="r")
                    s0 = sp.tile([C, FW], u8, tag="s")
                    for j in range(T0_SPLIT):
                        sl = slice(j * cw, (j + 1) * cw)
                        nc.vector.scalar_tensor_tensor(
                            r_prev[:, sl], m[:, sl], THRESH, m[:, sl], op.is_le, op.mult
                        )
                        nc.scalar.activation(
                            s0[:, sl], m[:, sl], AF.Sign, bias=bneg1[:], scale=1.0
                        )
                        nc.gpsimd.dma_start(out=out[:, 0, sl], in_=s0[:, sl])
                    continue

                m = mp.tile([C, FW], f32, tag="m")
                st = sp.tile([C, FW], u8, tag="s")
                if t == 1:
                    hw1 = FW // T1_SPLIT
                    r_new = rp.tile([C, FW], f32, tag="r")
                    for j in range(T1_SPLIT):
                        sl = slice(j * hw1, (j + 1) * hw1)
                        nc.vector.scalar_tensor_tensor(
                            m[:, sl], r_prev[:, sl], TAU, xts[t][:, sl], op.mult, op.add
                        )
                        nc.vector.scalar_tensor_tensor(
                            r_new[:, sl], m[:, sl], THRESH, m[:, sl], op.is_le, op.mult
                        )
                        nc.scalar.activation(
                            st[:, sl], m[:, sl], AF.Sign, bias=bneg1[:], scale=1.0
                        )
                        nc.gpsimd.dma_start(out=out[:, t, sl], in_=st[:, sl])
                    r_prev = r_new
                elif t == T - 1:
                    cw7 = FW // T7_SPLIT
                    for j in range(T7_SPLIT):
                        sl = slice(j * cw7, (j + 1) * cw7)
                        nc.vector.scalar_tensor_tensor(
                            m[:, sl], r_prev[:, sl], TAU, xts[t][:, sl], op.mult, op.add
                        )
                        nc.scalar.activation(
                            st[:, sl], m[:, sl], AF.Sign, bias=bneg1[:], scale=1.0
                        )
                        nc.gpsimd.dma_start(out=out[:, t, sl], in_=st[:, sl])
                else:
                    nc.vector.scalar_tensor_tensor(
                        m[:], r_prev[:], TAU, xts[t][:], op.mult, op.add
                    )
                    r_new = rp.tile([C, FW], f32, tag="r")
                    nc.vector.scalar_tensor_tensor(
                        r_new[:], m[:], THRESH, m[:], op.is_le, op.mult
                    )
                    nc.scalar.activation(
                        st[:], m[:], AF.Sign, bias=bneg1[:], scale=1.0
                    )
                    nc.gpsimd.dma_start(out=out[:, t], in_=st[:])
                    r_prev = r_new
    nc.compile()
    return nc


def make_in_maps(x: np.ndarray) -> list[dict]:
    # x [B, T, C, H, W] -> per core [C, T, B_loc*HW]
    xs = np.ascontiguousarray(x).reshape(B, T, C, HW)
    return [
        {
            "x": np.ascontiguousarray(
                xs[i * B_LOC : (i + 1) * B_LOC].transpose(2, 1, 0, 3)
            ).reshape(C, T, FW)
        }
        for i in range(N_CORES)
    ]


def kernel(x: np.ndarray) -> np.ndarray:
    global _nc_cache
    if _nc_cache is None:
        _nc_cache = build_nc()
    res = run_bass_kernel_spmd(_nc_cache, make_in_maps(x), list(range(N_CORES)))
    # out[c, t, b_loc*HW+hw] -> [b, t, c, hw]
    parts = [
        res.results[i]["out"].reshape(C, T, B_LOC, HW).transpose(2, 1, 0, 3)
        for i in range(N_CORES)
    ]
    full = np.concatenate(parts, axis=0)
    return full.reshape(B, T, C, H, W).astype(np.float32)
